# revision 48
# baseline (speedup 1.0000x reference)
"""Trainium2 Bass kernel for nn_MixedSparseSingleLayer (dense transformer layer
with LoRA adapters): RMSNorm -> QKV(+LoRA) -> RoPE -> causal attention ->
O-proj(+LoRA) -> residual -> RMSNorm -> MLP silu(up)+down (+LoRA) -> residual.

Sharding (8 NeuronCores): 2-way data parallel over batch x 4-way tensor
parallel (Megatron). Within a 4-core batch group:
  - norm1 is replicated (cheap), QKV is column-sharded so each core owns 4
    attention heads end-to-end (RoPE + causal softmax + PV).
  - attention outputs are exchanged with four per-head AllToAlls (0.5MB each,
    fired as each head finishes so the exchange hides behind attention
    compute) after which each core owns a 512-row slice for row-parallel
    O-proj + MLP.
LoRA (rank 16), biases and RMSNorm weights are folded on the host (exact
algebraic rewrites). Activations and weights are bf16 (PSUM accumulation is
fp32); the residual path stays fp32. v is produced directly in natural
[rows, hd] layout in the QKV phase so attention needs no PE transposes.
"""

import numpy as np
import ml_dtypes

import concourse.bass as bass
import concourse.mybir as mybir
import concourse.tile as tile
from concourse import bacc
from concourse.bass_utils import run_bass_kernel_spmd

f32 = mybir.dt.float32
f32r = mybir.dt.float32r
bf16 = mybir.dt.bfloat16

B, S, D, H, HD, F, R = 2, 2048, 2048, 16, 128, 8192, 16
P = 128
KD = D // P            # 16 d_model tiles
MQK = 8                # q|k output tiles of this core's 4 heads
NH = 4                 # heads per core
FT = F // P            # 64
ROWS = 512             # rows owned per core after the exchange (S / 4)
SCALE = 1.0 / float(np.sqrt(HD))
EPS = 1e-10

N_CORES = 8
GROUPS = [[0, 1, 2, 3, 4, 5, 6, 7]]
RH2 = ROWS // 2        # 256: rows owned per core per batch
NRH = 512              # rows per phase-A chunk
NRT = S // P           # 16 row tiles of v


def build_program(single_core=False):
    nc = bacc.Bacc(
        "TRN2",
        target_bir_lowering=False,
        debug=False,
        num_devices=1 if single_core else N_CORES,
    )

    # ---- I/O ----
    xbT_in = nc.dram_tensor("xbT", [D, S], bf16, kind="ExternalInput").ap()
    xrT_in = nc.dram_tensor("xrT", [D, ROWS], f32, kind="ExternalInput").ap()
    wqk_in = nc.dram_tensor("wqk", [MQK, P, KD, P], bf16, kind="ExternalInput").ap()
    bqk_in = nc.dram_tensor("bqk", [P, MQK], f32, kind="ExternalInput").ap()
    wv_in = nc.dram_tensor("wv", [P, KD * NH * P], bf16, kind="ExternalInput").ap()
    wo_in = nc.dram_tensor("wo", [KD, P, KD, P], bf16, kind="ExternalInput").ap()
    wup_in = nc.dram_tensor("wup", [FT, P, KD, P], bf16, kind="ExternalInput").ap()
    bup_in = nc.dram_tensor("bup", [P, FT], f32, kind="ExternalInput").ap()
    wdn_in = nc.dram_tensor("wdn", [KD, P, FT, P], bf16, kind="ExternalInput").ap()
    bdn_in = nc.dram_tensor("bdn", [P, KD], f32, kind="ExternalInput").ap()
    cosT_in = nc.dram_tensor("cosT", [P, S], bf16, kind="ExternalInput").ap()
    sinTs_in = nc.dram_tensor("sinTs", [P, S], bf16, kind="ExternalInput").ap()
    rotmT_in = nc.dram_tensor("rotmT", [P, P], bf16, kind="ExternalInput").ap()
    ones_in = nc.dram_tensor("ones", [P, P], f32r, kind="ExternalInput").ap()
    onesb_in = nc.dram_tensor("onesb", [P, P], bf16, kind="ExternalInput").ap()
    mask_in = nc.dram_tensor("mask", [P, 512], f32, kind="ExternalInput").ap()
    outT = nc.dram_tensor("outT", [D, ROWS], f32, kind="ExternalOutput").ap()

    with tile.TileContext(nc) as tc:
        _emit(tc, nc, xbT_in, xrT_in, wqk_in, bqk_in, wv_in, wo_in, wup_in,
              bup_in, wdn_in, bdn_in, cosT_in, sinTs_in, rotmT_in, ones_in,
              onesb_in, mask_in, outT, single_core)

    nc.compile()
    return nc


def _emit(tc, nc, xbT_in, xrT_in, wqk_in, bqk_in, wv_in, wo_in, wup_in,
          bup_in, wdn_in, bdn_in, cosT_in, sinTs_in, rotmT_in, ones_in,
          onesb_in, mask_in, outT, single_core=False):
    from contextlib import ExitStack

    top = ExitStack()
    with top:
        consts = top.enter_context(tc.tile_pool(name="consts", bufs=1))
        ones = consts.tile([P, P], f32r, tag="ones")
        nc.sync.dma_start(ones[:], ones_in)
        onesb = consts.tile([P, P], bf16, tag="onesb")
        nc.sync.dma_start(onesb[:], onesb_in)
        wmask_sb = consts.tile([P, 512], f32, tag="mask")
        nc.sync.dma_start(wmask_sb[:], mask_in)
        # cosT/sinTs/rotmT DMAs are issued at the end of phase A (the tiles
        # aren't needed until phase B, and the bytes would delay the x
        # chunk loads the PE is waiting on at startup)
        cosT = consts.tile([P, S], bf16, tag="cosT")
        sinTs = consts.tile([P, S], bf16, tag="sinTs")
        rotmT = consts.tile([P, P], bf16, tag="rotmT")
        bqk_sb = consts.tile([P, MQK], f32, tag="bqk")
        nc.sync.dma_start(bqk_sb[:], bqk_in)
        bup_sb = consts.tile([P, FT], f32, tag="bup")
        nc.sync.dma_start(bup_sb[:], bup_in)
        bdn_sb = consts.tile([P, KD], f32, tag="bdn")
        nc.sync.dma_start(bdn_sb[:], bdn_in)
        eps_sb = consts.tile([P, 1], f32, tag="eps")
        nc.vector.memset(eps_sb[:], EPS)
        # w_v resident in natural [d, vdim] layout (moving operand)
        wv_sb = consts.tile([P, KD * NH * P], bf16, tag="wv")

        # DRAM staging for two AllToAlls (head pairs 0-1 and 2-3; each
        # collective op costs ~35-50us of shared TOPSP control plane, so
        # fewer+bigger beats per-head). Chunk j of pair g holds this core's
        # heads 2g,2g+1 output (256 dims) x core j's 256 rows of THIS
        # core's batch; after the exchange a2a_out[g][j] row block hh is kd
        # tile 4*(j%4)+2g+hh of batch j//4 for this core's rows.
        dram = top.enter_context(tc.tile_pool(name="a2a", bufs=1, space="DRAM"))
        a2a_in = []
        a2a_out = []
        for g in range(NH // 2):
            a2a_in_g = dram.tile([N_CORES, 2 * P, RH2], bf16, tag=f"a2a_in{g}")
            a2a_out_g = dram.tile([N_CORES, 2 * P, RH2], bf16, tag=f"a2a_out{g}")
            a2a_in.append(a2a_in_g)
            a2a_out.append(a2a_out_g)

        # ================= Phase A: norm1 (replicated) + QK shard + V ======
        qkv_stack = ExitStack()
        qkvp = qkv_stack.enter_context(tc.tile_pool(name="qkT", bufs=1))
        qkT = qkvp.tile([P, MQK * S], bf16, tag="qkT")
        vnatp = qkv_stack.enter_context(tc.tile_pool(name="vnat", bufs=1))
        # natural v: [row-part, rt*512 + head*128 + hd]
        vnat = vnatp.tile([P, NRT * NH * P], bf16, tag="vnat")

        with tc.tile_pool(name="phA_sb", bufs=3) as pa, \
             tc.tile_pool(name="phA_sq", bufs=3) as sqp, \
             tc.tile_pool(name="phA_w", bufs=1) as wp, \
             tc.tile_pool(name="phA_ps", bufs=3, space="PSUM") as pps, \
             tc.tile_pool(name="phA_vps", bufs=2, space="PSUM") as vps, \
             tc.tile_pool(name="phA_st", bufs=1, space="PSUM") as stps, \
             tc.tile_pool(name="phA_r", bufs=2) as rp:
            # q/k weight tiles resident (32KB/partition); slices DMA'd
            # lazily inside chunk 0's mt loop so the first x chunk's bytes
            # win the DMA queues at startup
            wqk_sb = wp.tile([P, MQK * KD * P], bf16, tag="wqk")
            # small leading chunks so the first QKV matmuls start early
            chunks = [(0, 256), (256, 256), (512, 512), (1024, 512), (1536, 512)]
            for ci, (start, nr) in enumerate(chunks):
                xn1 = pa.tile([P, KD * nr], bf16, tag="xn1")
                for kd in range(KD):
                    nc.sync.dma_start(
                        xn1[:, kd * nr:(kd + 1) * nr],
                        xbT_in[kd * P:(kd + 1) * P, start:start + nr])
                if ci == 0:
                    nc.sync.dma_start(wv_sb[:], wv_in)
                # row stats: ssq[r] = sum_d x[d,r]^2  (PE ones-matmul trick);
                # squares on DVE, with the PE sum trailing one tile behind
                ssq = stps.tile([1, nr], f32, tag="ssq")
                sqs = [None] * KD
                for kd in range(KD):
                    sq = sqp.tile([P, nr], bf16, tag="sq")
                    nc.vector.tensor_mul(sq[:],
                                         xn1[:, kd * nr:(kd + 1) * nr],
                                         xn1[:, kd * nr:(kd + 1) * nr])
                    sqs[kd] = sq
                    if kd >= 1:
                        nc.tensor.matmul(
                            ssq[:], onesb[:, 0:1], sqs[kd - 1][:],
                            start=(kd == 1), stop=False)
                nc.tensor.matmul(ssq[:], onesb[:, 0:1], sqs[KD - 1][:],
                                 start=False, stop=True)
                sqr = rp.tile([1, nr], f32, tag="sqr")
                nc.scalar.activation(sqr[:], ssq[:],
                                     mybir.ActivationFunctionType.Sqrt,
                                     bias=eps_sb[0:1, :], scale=1.0 / D)
                rw = rp.tile([1, nr], f32, tag="rw")
                nc.vector.reciprocal_approx_fast(rw[:], sqr[:])
                rr = rp.tile([1, nr], f32, tag="rr")
                with nc.allow_low_precision(reason="f32r rounding for PE broadcast"):
                    nc.vector.tensor_copy(rr[:].bitcast(f32r), rw[:])
                rb = stps.tile([P, nr], f32, tag="rb")
                nc.tensor.matmul(rb[:], ones[0:1, :],
                                 rr[:].bitcast(f32r), start=True, stop=True)
                for kd in range(KD):
                    nc.vector.tensor_mul(xn1[:, kd * nr:(kd + 1) * nr],
                                         xn1[:, kd * nr:(kd + 1) * nr],
                                         rb[:])
                # v in natural layout: out rows = tokens (stationary is the
                # activation slice), moving = w_v columns for all 4 heads
                for rt in range(nr // P):
                    vacc = vps.tile([P, NH * P], f32, tag="vacc")
                    for kd in range(KD):
                        nc.tensor.matmul(
                            vacc[:],
                            xn1[:, kd * nr + rt * P: kd * nr + (rt + 1) * P],
                            wv_sb[:, kd * NH * P:(kd + 1) * NH * P],
                            start=(kd == 0), stop=(kd == KD - 1))
                    grt = start // P + rt
                    nc.scalar.copy(
                        vnat[:, grt * NH * P:(grt + 1) * NH * P], vacc[:])
                # q,k matmuls: head-major m order so RoPE can start early
                for mt in (0, 4, 1, 5, 2, 6, 3, 7):
                    if ci == 0:
                        nc.sync.dma_start(
                            wqk_sb[:, mt * KD * P:(mt + 1) * KD * P],
                            wqk_in[mt].rearrange("p k m -> p (k m)"))
                    acc = pps.tile([P, nr], f32, tag="qkacc")
                    for kd in range(KD):
                        nc.tensor.matmul(
                            acc[:],
                            wqk_sb[:, (mt * KD + kd) * P:(mt * KD + kd + 1) * P],
                            xn1[:, kd * nr:(kd + 1) * nr],
                            start=(kd == 0), stop=(kd == KD - 1))
                    nc.scalar.activation(
                        qkT[:, mt * S + start: mt * S + start + nr],
                        acc[:], mybir.ActivationFunctionType.Identity,
                        bias=bqk_sb[:, mt:mt + 1])
            # rope constants, needed from phase B on
            nc.sync.dma_start(cosT[:], cosT_in)
            nc.sync.dma_start(sinTs[:], sinTs_in)
            nc.sync.dma_start(rotmT[:], rotmT_in)

        # ================= Phase B: attention (4 heads) =====================
        with tc.tile_pool(name="rope", bufs=2) as ropep, \
             tc.tile_pool(name="rtmp", bufs=2) as rtmpp, \
             tc.tile_pool(name="prT", bufs=4) as prtp, \
             tc.tile_pool(name="lsum", bufs=4) as lp, \
             tc.tile_pool(name="rbc", bufs=2) as rbcp, \
             tc.tile_pool(name="oT", bufs=2) as otp, \
             tc.tile_pool(name="sc_ps", bufs=2, space="PSUM") as scps, \
             tc.tile_pool(name="ov_ps", bufs=2, space="PSUM") as ovps, \
             tc.tile_pool(name="st_ps", bufs=2, space="PSUM") as stp2:
            def emit_norm(qc, opsum, lps, h, oTh):
                rw = lp.tile([1, 512], f32, tag="rinvw")
                nc.vector.reciprocal_approx_fast(rw[:], lps[:])
                rinv = lp.tile([1, 512], f32, tag="rinv")
                with nc.allow_low_precision(reason="f32r rounding for PE bcast"):
                    nc.vector.tensor_copy(rinv[:].bitcast(f32r), rw[:])
                rbc = scps.tile([P, 1024], f32, tag="scc")
                nc.tensor.matmul(rbc[:, 0:512], ones[0:1, :],
                                 rinv[:].bitcast(f32r),
                                 start=True, stop=True)
                rbs = rbcp.tile([P, 512], f32, tag="rbs")
                nc.vector.tensor_copy(rbs[:], rbc[:, 0:512])
                nc.vector.tensor_mul(oTh[:, qc * 512:(qc + 1) * 512],
                                     opsum[:], rbs[:])
                # stage the finished 256-column chunks for the AllToAll
                hh = h % 2
                for j in (2 * qc, 2 * qc + 1):
                    nc.sync.dma_start(
                        a2a_in[h // 2][j, hh * P:(hh + 1) * P, :],
                        oTh[:, j * RH2:(j + 1) * RH2])

            def fire_norm(pend):
                qc, opsum, lps, h, oTh, last = pend
                emit_norm(qc, opsum, lps, h, oTh)
                if not last:
                    return
                # head pair fully staged -> fire its AllToAll
                g = h // 2
                if single_core:
                    # timing-only stand-in for the collective (cost-model
                    # sim has no multi-core support)
                    nc.sync.dma_start(
                        a2a_out[g][:].rearrange("a r c -> (a r) c"),
                        a2a_in[g][:].rearrange("a r c -> (a r) c"))
                else:
                    nc.gpsimd.collective_compute(
                        "AllToAll", mybir.AluOpType.bypass,
                        replica_groups=GROUPS,
                        ins=[a2a_in[g][:].opt()],
                        outs=[a2a_out[g][:].opt()],
                    )

            pend_norm = None
            for h in range(NH):
                q_sl = qkT[:, h * S:(h + 1) * S]
                k_sl = qkT[:, (NH + h) * S:(NH + h + 1) * S]
                # RoPE on q and k. rotate_half is a cross-partition shuffle,
                # which DVE lanes cannot do, so apply it as a PE matmul with
                # a signed permutation matrix (sign of rotate_half baked in).
                rq = ropep.tile([P, S], bf16, tag="ropeq")
                rk = ropep.tile([P, S], bf16, tag="ropek")
                # per-512-chunk, k before q, so scores for q-chunk qc only
                # wait on rope chunks <= qc instead of the whole tensor
                for c in range(S // 512):
                    sl = slice(c * 512, (c + 1) * 512)
                    for src, dst in ((k_sl, rk), (q_sl, rq)):
                        tmp = rtmpp.tile([P, 512], bf16, tag="rtmp")
                        rt = scps.tile([P, 512], f32, tag="scc")
                        nc.tensor.matmul(
                            rt[:], rotmT[:], src[:, sl],
                            start=True, stop=True)
                        nc.vector.tensor_mul(tmp[:], rt[:], sinTs[:, sl])
                        nc.vector.tensor_mul(dst[:, sl], src[:, sl],
                                             cosT[:, sl])
                        nc.vector.tensor_add(dst[:, sl], dst[:, sl], tmp[:])
                oTh = otp.tile([P, S], bf16, tag="oTh")
                # q processed in 512-wide chunks; scores computed TRANSPOSED
                # (s.T[S_k, q]) so exp output is already in PV layout.
                # kt tiles are processed in PAIRS sharing one wide exp (the
                # ACT exp cadence, not the PE, limits this loop otherwise);
                # lps/opsum trail the scc/exp front by SKEWP pairs, and the
                # softmax normalization of chunk qc is deferred into chunk
                # qc+1 so the reciprocal latency hides behind matmuls.
                SKEWP = 2
                for qc in range(S // 512):
                    opsum = ovps.tile([P, 512], f32, tag="opv")
                    lps = stp2.tile([1, 512], f32, tag="lps")
                    npair = (4 * qc + 4) // 2
                    prts = [None] * npair

                    def front(pi, qc=qc, prts=prts):
                        scc = scps.tile([P, 1024], f32, tag="scc")
                        for half in range(2):
                            kt = 2 * pi + half
                            nc.tensor.matmul(
                                scc[:, half * 512:(half + 1) * 512],
                                rk[:, kt * P:(kt + 1) * P],
                                rq[:, qc * 512:(qc + 1) * 512],
                                start=True, stop=True)
                            lb = kt - 4 * qc
                            if lb >= 0:
                                # mask: lb full 128-blocks + triangular
                                nc.vector.tensor_add(
                                    scc[:, half * 512:half * 512 + (lb + 1) * P],
                                    scc[:, half * 512:half * 512 + (lb + 1) * P],
                                    wmask_sb[:, (3 - lb) * P:512])
                        prT = prtp.tile([P, 1024], bf16, tag="prT")
                        nc.scalar.activation(
                            prT[:], scc[:],
                            mybir.ActivationFunctionType.Exp, scale=SCALE)
                        prts[pi] = prT

                    def back(pi, npair=npair, lps=lps, opsum=opsum, h=h,
                             prts=prts):
                        prT = prts[pi]
                        for half in range(2):
                            kt = 2 * pi + half
                            nc.tensor.matmul(
                                lps[:], onesb[:, 0:1],
                                prT[:, half * 512:(half + 1) * 512],
                                start=(pi == 0 and half == 0),
                                stop=(pi == npair - 1 and half == 1))
                            nc.tensor.matmul(
                                opsum[:],
                                vnat[:, kt * NH * P + h * P:
                                     kt * NH * P + (h + 1) * P],
                                prT[:, half * 512:(half + 1) * 512],
                                start=(pi == 0 and half == 0),
                                stop=(pi == npair - 1 and half == 1))

                    for pi in range(npair):
                        front(pi)
                        if pi == SKEWP and pend_norm is not None:
                            fire_norm(pend_norm)
                            pend_norm = None
                        if pi >= SKEWP:
                            back(pi - SKEWP)
                    if pend_norm is not None:
                        fire_norm(pend_norm)
                        pend_norm = None
                    for pi in range(max(0, npair - SKEWP), npair):
                        back(pi)
                    pend_norm = (qc, opsum, lps, h, oTh,
                                 qc == S // 512 - 1 and h % 2 == 1)
            fire_norm(pend_norm)
            pend_norm = None
        qkv_stack.close()

        # ================= Phase C..F: row-parallel O-proj + MLP ============
        x1_stack = ExitStack()
        x1p = x1_stack.enter_context(tc.tile_pool(name="x1T", bufs=1))
        x1T = x1p.tile([P, KD * ROWS], f32, tag="x1T")
        omp = x1_stack.enter_context(tc.tile_pool(name="phC_om", bufs=1))
        oT_mine = omp.tile([P, KD * ROWS], bf16, tag="oT_mine")
        dps = x1_stack.enter_context(
            tc.tile_pool(name="phC_st", bufs=1, space="PSUM"))
        rp2 = x1_stack.enter_context(tc.tile_pool(name="phD_r", bufs=2))
        # pull the exchanged chunks into SBUF as they land
        for h in range(NH):
            for j in range(N_CORES):
                b, g = divmod(j, 4)
                kd = 4 * g + h
                nc.sync.dma_start(
                    oT_mine[:, kd * ROWS + b * RH2: kd * ROWS + (b + 1) * RH2],
                    a2a_out[h // 2][j, (h % 2) * P:(h % 2 + 1) * P, :])

        # k-tile order, h-major (matches the host-side permutation of wo's
        # k axis): tiles for head h arrive with AllToAll h
        PERM = [4 * g + h for h in range(NH) for g in range(NH)]
        with tc.tile_pool(name="phC_xr", bufs=3) as xrp, \
             tc.tile_pool(name="phC_w", bufs=3) as wop, \
             tc.tile_pool(name="phC_o1", bufs=1) as o1p, \
             tc.tile_pool(name="phC_sq", bufs=2) as sqp2, \
             tc.tile_pool(name="phC_ps", bufs=3, space="PSUM") as cps, \
             tc.tile_pool(name="phC_ps2", bufs=3, space="PSUM") as cps2:
            ssq2 = dps.tile([1, ROWS], f32, tag="ssq2")
            o1 = o1p.tile([P, KD * ROWS], f32, tag="o1")
            # pass 1: the 8 k-tiles from heads 0-1 — runnable while the
            # second AllToAll is still in flight
            NP1 = 8
            for mt in range(KD):
                wsb = wop.tile([P, NP1 * P], bf16, tag="wo1")
                nc.sync.dma_start(
                    wsb[:],
                    wo_in[mt][:, 0:NP1, :].rearrange("p k m -> p (k m)"))
                acc = cps.tile([P, ROWS], f32, tag="oacc1")
                for i in range(NP1):
                    kd = PERM[i]
                    nc.tensor.matmul(
                        acc[:], wsb[:, i * P:(i + 1) * P],
                        oT_mine[:, kd * ROWS:(kd + 1) * ROWS],
                        start=(i == 0), stop=(i == NP1 - 1))
                xr = xrp.tile([P, ROWS], f32, tag="xr")
                nc.sync.dma_start(xr[:], xrT_in[mt * P:(mt + 1) * P, :])
                # fold the residual in now, off the pass-2 critical path
                nc.vector.tensor_add(o1[:, mt * ROWS:(mt + 1) * ROWS],
                                     acc[:], xr[:])
            # pass 2: head 3's 4 k-tiles + residual + norm2 stats (squares on
            # DVE, PE sum trailing one tile)
            sq2s = [None] * KD
            for mt in range(KD):
                wsb = wop.tile([P, (KD - NP1) * P], bf16, tag="wo2")
                nc.sync.dma_start(
                    wsb[:],
                    wo_in[mt][:, NP1:KD, :].rearrange("p k m -> p (k m)"))
                acc = cps2.tile([P, ROWS], f32, tag="oacc2")
                for i in range(KD - NP1):
                    kd = PERM[NP1 + i]
                    nc.tensor.matmul(
                        acc[:], wsb[:, i * P:(i + 1) * P],
                        oT_mine[:, kd * ROWS:(kd + 1) * ROWS],
                        start=(i == 0), stop=(i == KD - NP1 - 1))
                nc.vector.tensor_add(x1T[:, mt * ROWS:(mt + 1) * ROWS],
                                     acc[:], o1[:, mt * ROWS:(mt + 1) * ROWS])
                sq = sqp2.tile([P, ROWS], bf16, tag="sq2")
                nc.scalar.activation(sq[:], x1T[:, mt * ROWS:(mt + 1) * ROWS],
                                     mybir.ActivationFunctionType.Square)
                sq2s[mt] = sq
                if mt >= 1:
                    nc.tensor.matmul(ssq2[:], onesb[:, 0:1], sq2s[mt - 1][:],
                                     start=(mt == 1), stop=False)
            nc.tensor.matmul(ssq2[:], onesb[:, 0:1], sq2s[KD - 1][:],
                             start=False, stop=True)

        sqr2 = rp2.tile([1, ROWS], f32, tag="sqr2")
        nc.scalar.activation(sqr2[:], ssq2[:],
                             mybir.ActivationFunctionType.Sqrt,
                             bias=eps_sb[0:1, :], scale=1.0 / D)
        rw2 = rp2.tile([1, ROWS], f32, tag="rw2")
        nc.vector.reciprocal_approx_fast(rw2[:], sqr2[:])
        rr2 = rp2.tile([1, ROWS], f32, tag="rr2")
        with nc.allow_low_precision(reason="f32r rounding for PE broadcast"):
            nc.vector.tensor_copy(rr2[:].bitcast(f32r), rw2[:])
        rb2 = dps.tile([P, ROWS], f32, tag="rb2")
        nc.tensor.matmul(rb2[:], ones[0:1, :],
                         rr2[:].bitcast(f32r), start=True, stop=True)

        mlp_stack = ExitStack()
        xn2p = mlp_stack.enter_context(tc.tile_pool(name="xn2", bufs=1))
        fnp = mlp_stack.enter_context(tc.tile_pool(name="fnT", bufs=1))
        xn2 = xn2p.tile([P, KD * ROWS], bf16, tag="xn2")
        fnT = fnp.tile([P, FT * ROWS], bf16, tag="fnT")
        for kd in range(KD):
            nc.vector.tensor_mul(xn2[:, kd * ROWS:(kd + 1) * ROWS],
                                 x1T[:, kd * ROWS:(kd + 1) * ROWS], rb2[:])

        with tc.tile_pool(name="phE_w", bufs=3) as wup_p, \
             tc.tile_pool(name="phE_sig", bufs=2) as sigp, \
             tc.tile_pool(name="phE_ps", bufs=4, space="PSUM") as eps_ps:
            for mt in range(FT):
                wsb = wup_p.tile([P, KD * P], bf16, tag="wup")
                nc.sync.dma_start(wsb[:], wup_in[mt].rearrange("p k m -> p (k m)"))
                acc = eps_ps.tile([P, ROWS], f32, tag="upacc")
                for kd in range(KD):
                    nc.tensor.matmul(
                        acc[:], wsb[:, kd * P:(kd + 1) * P],
                        xn2[:, kd * ROWS:(kd + 1) * ROWS],
                        start=(kd == 0), stop=(kd == KD - 1))
                sig = sigp.tile([P, ROWS], f32, tag="sig")
                nc.scalar.activation(sig[:], acc[:],
                                     mybir.ActivationFunctionType.Sigmoid,
                                     bias=bup_sb[:, mt:mt + 1])
                # fn = (up + b_up) * sigmoid(up + b_up), cast to bf16
                nc.vector.scalar_tensor_tensor(
                    fnT[:, mt * ROWS:(mt + 1) * ROWS], acc[:],
                    bup_sb[:, mt:mt + 1], sig[:],
                    op0=mybir.AluOpType.add, op1=mybir.AluOpType.mult)

        # down-proj: half-size weight tiles so the prefetch pipelines
        with tc.tile_pool(name="phF_w", bufs=3) as wdn_p, \
             tc.tile_pool(name="phF_out", bufs=2) as outp, \
             tc.tile_pool(name="phF_ps", bufs=4, space="PSUM") as fps:
            FH = FT // 2
            for mt in range(KD):
                acc = fps.tile([P, ROWS], f32, tag="dnacc")
                for half in range(2):
                    wsb = wdn_p.tile([P, FH * P], bf16, tag="wdn")
                    nc.sync.dma_start(
                        wsb[:],
                        wdn_in[mt][:, half * FH:(half + 1) * FH, :]
                        .rearrange("p k m -> p (k m)"))
                    for kt in range(FH):
                        kd = half * FH + kt
                        nc.tensor.matmul(
                            acc[:], wsb[:, kt * P:(kt + 1) * P],
                            fnT[:, kd * ROWS:(kd + 1) * ROWS],
                            start=(kd == 0), stop=(kd == FT - 1))
                out_sb = outp.tile([P, ROWS], f32, tag="out_sb")
                nc.vector.scalar_tensor_tensor(
                    out_sb[:], acc[:], bdn_sb[:, mt:mt + 1],
                    x1T[:, mt * ROWS:(mt + 1) * ROWS],
                    op0=mybir.AluOpType.add, op1=mybir.AluOpType.add)
                nc.sync.dma_start(outT[mt * P:(mt + 1) * P, :], out_sb[:])
        mlp_stack.close()
        x1_stack.close()


def host_prepare(inputs):
    """Fold LoRA/norm-weights/biases and build the 8 per-core input maps."""
    gi = {k: np.asarray(v, dtype=np.float32) if np.asarray(v).dtype != np.float32
          else np.asarray(v) for k, v in inputs.items()}

    def fold(nm):
        return gi['w_' + nm] + gi['w_' + nm + '_lora_a'] @ gi['w_' + nm + '_lora_b']

    nw1 = gi['norm_weight_1'][:, None]
    nw2 = gi['norm_weight_2'][:, None]
    w_q = (nw1 * fold('q')).astype(np.float32)
    w_k = (nw1 * fold('k')).astype(np.float32)
    w_v = (nw1 * fold('v')).astype(np.float32)
    w_o = fold('o').astype(np.float32)
    w_up = (nw2 * fold('up')).astype(np.float32)
    w_dn = fold('down').astype(np.float32)

    # pre-tiled weight layouts; wo's k axis is permuted h-major to match the
    # kernel's two-pass O-proj (tiles for head h arrive with AllToAll h)
    perm = [4 * g + h for h in range(4) for g in range(4)]
    wo_t = np.ascontiguousarray(
        w_o.reshape(KD, P, KD, P).transpose(2, 1, 0, 3)[:, :, perm, :]
    ).astype(ml_dtypes.bfloat16)
    wup_t = np.ascontiguousarray(
        w_up.reshape(KD, P, FT, P).transpose(2, 1, 0, 3)).astype(ml_dtypes.bfloat16)
    wdn_t = np.ascontiguousarray(
        w_dn.reshape(FT, P, KD, P).transpose(2, 1, 0, 3)).astype(ml_dtypes.bfloat16)
    bup_t = np.ascontiguousarray(gi['b_up'].reshape(FT, P).T)
    bdn_t = np.ascontiguousarray(gi['b_down'].reshape(KD, P).T)

    cosT = np.ascontiguousarray(gi['cos'].T).astype(ml_dtypes.bfloat16)
    sinTs = np.ascontiguousarray(gi['sin'].T).astype(ml_dtypes.bfloat16)
    # rot(x).T = R @ x.T with R[d, d+64] = -1 (d<64), R[d, d-64] = +1;
    # matmul computes lhsT.T @ rhs, so pass R.T.
    Rm = np.zeros((P, P), dtype=np.float32)
    hh = HD // 2
    Rm[np.arange(hh), np.arange(hh) + hh] = -1.0
    Rm[np.arange(hh) + hh, np.arange(hh)] = 1.0
    rotmT = np.ascontiguousarray(Rm.T).astype(ml_dtypes.bfloat16)
    maskT = np.maximum(gi['attention_mask'][0, 0, :P, :P], -2000.0).T
    wmask = np.full((P, 512), -2000.0, dtype=np.float32)
    wmask[:, 384:512] = maskT
    mask128 = np.ascontiguousarray(wmask)

    x = gi['x']
    # b_v and b_o folded into the residual: softmax rows sum to exactly 1,
    # so o_final = o @ w_o + (b_v @ w_o + b_o)
    radd = gi['b_v'] @ w_o + gi['b_o']

    # per-group (4-way TP) weight shards, shared across the two batches
    wqk_t, bqk_t, wv_t = [], [], []
    for g in range(4):
        hs = slice(512 * g, 512 * (g + 1))
        wqk = np.concatenate([w_q[:, hs], w_k[:, hs]], axis=1)
        wqk_t.append(np.ascontiguousarray(
            wqk.reshape(KD, P, MQK, P).transpose(2, 1, 0, 3)).astype(ml_dtypes.bfloat16))
        bqk = np.concatenate([gi['b_q'][hs], gi['b_k'][hs]])
        bqk_t.append(np.ascontiguousarray(bqk.reshape(MQK, P).T))
        wv_t.append(np.ascontiguousarray(
            w_v[:, hs].reshape(KD, P, NH * P).transpose(1, 0, 2)
            .reshape(P, KD * NH * P)).astype(ml_dtypes.bfloat16))

    xT = [np.ascontiguousarray(x[b].T).astype(ml_dtypes.bfloat16) for b in range(B)]

    ones = np.ones((P, P), dtype=np.float32)
    onesb = np.ones((P, P), dtype=ml_dtypes.bfloat16)

    in_maps = []
    for i in range(N_CORES):
        b, g = divmod(i, 4)
        # this core owns rows [256i, 256(i+1)) of BOTH batches
        xrows = np.concatenate(
            [x[0, RH2 * i:RH2 * (i + 1)], x[1, RH2 * i:RH2 * (i + 1)]], axis=0)
        xrT = np.ascontiguousarray(xrows.T + radd[:, None])
        in_maps.append({
            "xbT": xT[b], "xrT": xrT,
            "wqk": wqk_t[g], "bqk": bqk_t[g], "wv": wv_t[g],
            "wo": wo_t, "wup": wup_t, "bup": bup_t,
            "wdn": wdn_t, "bdn": bdn_t,
            "cosT": cosT, "sinTs": sinTs, "rotmT": rotmT,
            "ones": ones, "onesb": onesb, "mask": mask128,
        })
    return in_maps


def assemble(results):
    out = np.empty((B, S, D), dtype=np.float32)
    for i in range(N_CORES):
        oT = results[i]["outT"]
        out[0, RH2 * i:RH2 * (i + 1), :] = oT[:, 0:RH2].T
        out[1, RH2 * i:RH2 * (i + 1), :] = oT[:, RH2:ROWS].T
    return out


_NC_CACHE = {}


def get_nc():
    if "nc" not in _NC_CACHE:
        _NC_CACHE["nc"] = build_program()
    return _NC_CACHE["nc"]


def kernel(**inputs):
    nc = get_nc()
    in_maps = host_prepare(inputs)
    res = run_bass_kernel_spmd(nc, in_maps, list(range(N_CORES)))
    return assemble(res.results)


# revision 54
# speedup vs baseline: 1.0705x; 1.0705x over previous
"""Trainium2 Bass kernel for nn_MixedSparseSingleLayer (dense transformer layer
with LoRA adapters): RMSNorm -> QKV(+LoRA) -> RoPE -> causal attention ->
O-proj(+LoRA) -> residual -> RMSNorm -> MLP silu(up)+down (+LoRA) -> residual.

Sharding (8 NeuronCores): 2-way data parallel over batch x 4-way tensor
parallel (Megatron). Within a 4-core batch group:
  - norm1 is replicated (cheap), QKV is column-sharded so each core owns 4
    attention heads end-to-end (RoPE + causal softmax + PV).
  - attention outputs are exchanged with four per-head AllToAlls (0.5MB each,
    fired as each head finishes so the exchange hides behind attention
    compute) after which each core owns a 512-row slice for row-parallel
    O-proj + MLP.
LoRA (rank 16), biases and RMSNorm weights are folded on the host (exact
algebraic rewrites). Activations and weights are bf16 (PSUM accumulation is
fp32); the residual path stays fp32. v is produced directly in natural
[rows, hd] layout in the QKV phase so attention needs no PE transposes.
"""

import numpy as np
import ml_dtypes

import concourse.bass as bass
import concourse.mybir as mybir
import concourse.tile as tile
from concourse import bacc
from concourse.bass_utils import run_bass_kernel_spmd

f32 = mybir.dt.float32
f32r = mybir.dt.float32r
bf16 = mybir.dt.bfloat16

B, S, D, H, HD, F, R = 2, 2048, 2048, 16, 128, 8192, 16
P = 128
KD = D // P            # 16 d_model tiles
MQK = 8                # q|k output tiles of this core's 4 heads
NH = 4                 # heads per core
FT = F // P            # 64
ROWS = 512             # rows owned per core after the exchange (S / 4)
SCALE = 1.0 / float(np.sqrt(HD))
EPS = 1e-10

N_CORES = 8
GROUPS = [[0, 1, 2, 3, 4, 5, 6, 7]]
RH2 = ROWS // 2        # 256: rows owned per core per batch
NRH = 512              # rows per phase-A chunk
NRT = S // P           # 16 row tiles of v


def build_program(single_core=False):
    nc = bacc.Bacc(
        "TRN2",
        target_bir_lowering=False,
        debug=False,
        num_devices=1 if single_core else N_CORES,
    )

    # ---- I/O ----
    xbT_in = nc.dram_tensor("xbT", [D, S], bf16, kind="ExternalInput").ap()
    xrT_in = nc.dram_tensor("xrT", [D, ROWS], f32, kind="ExternalInput").ap()
    wqk_in = nc.dram_tensor("wqk", [MQK, P, KD, P], bf16, kind="ExternalInput").ap()
    bqk_in = nc.dram_tensor("bqk", [P, MQK], f32, kind="ExternalInput").ap()
    wv_in = nc.dram_tensor("wv", [P, KD * NH * P], bf16, kind="ExternalInput").ap()
    wo_in = nc.dram_tensor("wo", [KD, P, KD, P], bf16, kind="ExternalInput").ap()
    wup_in = nc.dram_tensor("wup", [FT, P, KD, P], bf16, kind="ExternalInput").ap()
    bup_in = nc.dram_tensor("bup", [P, FT], f32, kind="ExternalInput").ap()
    wdn_in = nc.dram_tensor("wdn", [KD, P, FT, P], bf16, kind="ExternalInput").ap()
    bdn_in = nc.dram_tensor("bdn", [P, KD], f32, kind="ExternalInput").ap()
    cosT_in = nc.dram_tensor("cosT", [P, S], bf16, kind="ExternalInput").ap()
    sinTs_in = nc.dram_tensor("sinTs", [P, S], bf16, kind="ExternalInput").ap()
    rotmT_in = nc.dram_tensor("rotmT", [P, P], bf16, kind="ExternalInput").ap()
    ones_in = nc.dram_tensor("ones", [P, P], f32r, kind="ExternalInput").ap()
    onesb_in = nc.dram_tensor("onesb", [P, P], bf16, kind="ExternalInput").ap()
    mask_in = nc.dram_tensor("mask", [P, 512], f32, kind="ExternalInput").ap()
    outT = nc.dram_tensor("outT", [D, ROWS], f32, kind="ExternalOutput").ap()

    with tile.TileContext(nc) as tc:
        _emit(tc, nc, xbT_in, xrT_in, wqk_in, bqk_in, wv_in, wo_in, wup_in,
              bup_in, wdn_in, bdn_in, cosT_in, sinTs_in, rotmT_in, ones_in,
              onesb_in, mask_in, outT, single_core)

    nc.compile()
    return nc


def _emit(tc, nc, xbT_in, xrT_in, wqk_in, bqk_in, wv_in, wo_in, wup_in,
          bup_in, wdn_in, bdn_in, cosT_in, sinTs_in, rotmT_in, ones_in,
          onesb_in, mask_in, outT, single_core=False):
    from contextlib import ExitStack

    top = ExitStack()
    with top:
        consts = top.enter_context(tc.tile_pool(name="consts", bufs=1))
        ones = consts.tile([P, P], f32r, tag="ones")
        nc.sync.dma_start(ones[:], ones_in)
        onesb = consts.tile([P, P], bf16, tag="onesb")
        nc.sync.dma_start(onesb[:], onesb_in)
        wmask_sb = consts.tile([P, 512], f32, tag="mask")
        nc.sync.dma_start(wmask_sb[:], mask_in)
        # cosT/sinTs/rotmT DMAs are issued at the end of phase A (the tiles
        # aren't needed until phase B, and the bytes would delay the x
        # chunk loads the PE is waiting on at startup)
        cosT = consts.tile([P, S], bf16, tag="cosT")
        sinTs = consts.tile([P, S], bf16, tag="sinTs")
        rotmT = consts.tile([P, P], bf16, tag="rotmT")
        bqk_sb = consts.tile([P, MQK], f32, tag="bqk")
        nc.sync.dma_start(bqk_sb[:], bqk_in)
        bup_sb = consts.tile([P, FT], f32, tag="bup")
        nc.sync.dma_start(bup_sb[:], bup_in)
        bdn_sb = consts.tile([P, KD], f32, tag="bdn")
        nc.sync.dma_start(bdn_sb[:], bdn_in)
        eps_sb = consts.tile([P, 1], f32, tag="eps")
        nc.vector.memset(eps_sb[:], EPS)
        # w_v resident in natural [d, vdim] layout (moving operand)
        wv_sb = consts.tile([P, KD * NH * P], bf16, tag="wv")

        # DRAM staging for the four per-head AllToAlls. Chunk j of head h
        # holds this core's head-h output (128 dims) x core j's 256 rows of
        # THIS core's batch; after the exchange a2a_out[h][j] is kd tile
        # 4*(j%4)+h of batch j//4 for this core's rows. The first collective
        # op after the barrier pays a ~90us warmup, so a 2KB dummy AllToAll
        # is fired during phase A to absorb it.
        dram = top.enter_context(tc.tile_pool(name="a2a", bufs=1, space="DRAM"))
        a2a_in = []
        a2a_out = []
        for h in range(NH):
            a2a_in_h = dram.tile([N_CORES, P, RH2], bf16, tag=f"a2a_in{h}")
            a2a_out_h = dram.tile([N_CORES, P, RH2], bf16, tag=f"a2a_out{h}")
            a2a_in.append(a2a_in_h)
            a2a_out.append(a2a_out_h)
        warm_in = dram.tile([N_CORES, 1, P], bf16, tag="warm_in")
        warm_out = dram.tile([N_CORES, 1, P], bf16, tag="warm_out")
        nc.sync.dma_start(warm_in[:].rearrange("a r c -> (a r) c"), onesb_in[0:8, :])
        if not single_core:
            nc.gpsimd.collective_compute(
                "AllToAll", mybir.AluOpType.bypass,
                replica_groups=GROUPS,
                ins=[warm_in[:].opt()],
                outs=[warm_out[:].opt()],
            )

        # ================= Phase A: norm1 (replicated) + QK shard + V ======
        qkv_stack = ExitStack()
        qkvp = qkv_stack.enter_context(tc.tile_pool(name="qkT", bufs=1))
        qkT = qkvp.tile([P, MQK * S], bf16, tag="qkT")
        vnatp = qkv_stack.enter_context(tc.tile_pool(name="vnat", bufs=1))
        # natural v: [row-part, rt*512 + head*128 + hd]
        vnat = vnatp.tile([P, NRT * NH * P], bf16, tag="vnat")

        with tc.tile_pool(name="phA_sb", bufs=3) as pa, \
             tc.tile_pool(name="phA_sq", bufs=3) as sqp, \
             tc.tile_pool(name="phA_w", bufs=1) as wp, \
             tc.tile_pool(name="phA_ps", bufs=3, space="PSUM") as pps, \
             tc.tile_pool(name="phA_vps", bufs=2, space="PSUM") as vps, \
             tc.tile_pool(name="phA_st", bufs=1, space="PSUM") as stps, \
             tc.tile_pool(name="phA_r", bufs=2) as rp:
            # q/k weight tiles resident (32KB/partition); slices DMA'd
            # lazily inside chunk 0's mt loop so the first x chunk's bytes
            # win the DMA queues at startup
            wqk_sb = wp.tile([P, MQK * KD * P], bf16, tag="wqk")
            # small leading chunks so the first QKV matmuls start early
            chunks = [(0, 256), (256, 256), (512, 512), (1024, 512), (1536, 512)]
            for ci, (start, nr) in enumerate(chunks):
                xn1 = pa.tile([P, KD * nr], bf16, tag="xn1")
                for kd in range(KD):
                    nc.sync.dma_start(
                        xn1[:, kd * nr:(kd + 1) * nr],
                        xbT_in[kd * P:(kd + 1) * P, start:start + nr])
                if ci == 0:
                    nc.sync.dma_start(wv_sb[:], wv_in)
                # row stats: ssq[r] = sum_d x[d,r]^2  (PE ones-matmul trick);
                # squares on DVE, with the PE sum trailing one tile behind
                ssq = stps.tile([1, nr], f32, tag="ssq")
                sqs = [None] * KD
                for kd in range(KD):
                    sq = sqp.tile([P, nr], bf16, tag="sq")
                    nc.vector.tensor_mul(sq[:],
                                         xn1[:, kd * nr:(kd + 1) * nr],
                                         xn1[:, kd * nr:(kd + 1) * nr])
                    sqs[kd] = sq
                    if kd >= 1:
                        nc.tensor.matmul(
                            ssq[:], onesb[:, 0:1], sqs[kd - 1][:],
                            start=(kd == 1), stop=False)
                nc.tensor.matmul(ssq[:], onesb[:, 0:1], sqs[KD - 1][:],
                                 start=False, stop=True)
                sqr = rp.tile([1, nr], f32, tag="sqr")
                nc.scalar.activation(sqr[:], ssq[:],
                                     mybir.ActivationFunctionType.Sqrt,
                                     bias=eps_sb[0:1, :], scale=1.0 / D)
                rw = rp.tile([1, nr], f32, tag="rw")
                nc.vector.reciprocal_approx_fast(rw[:], sqr[:])
                rr = rp.tile([1, nr], f32, tag="rr")
                with nc.allow_low_precision(reason="f32r rounding for PE broadcast"):
                    nc.vector.tensor_copy(rr[:].bitcast(f32r), rw[:])
                rb = stps.tile([P, nr], f32, tag="rb")
                nc.tensor.matmul(rb[:], ones[0:1, :],
                                 rr[:].bitcast(f32r), start=True, stop=True)
                for kd in range(KD):
                    nc.vector.tensor_mul(xn1[:, kd * nr:(kd + 1) * nr],
                                         xn1[:, kd * nr:(kd + 1) * nr],
                                         rb[:])
                # v in natural layout: out rows = tokens (stationary is the
                # activation slice), moving = w_v columns for all 4 heads
                for rt in range(nr // P):
                    vacc = vps.tile([P, NH * P], f32, tag="vacc")
                    for kd in range(KD):
                        nc.tensor.matmul(
                            vacc[:],
                            xn1[:, kd * nr + rt * P: kd * nr + (rt + 1) * P],
                            wv_sb[:, kd * NH * P:(kd + 1) * NH * P],
                            start=(kd == 0), stop=(kd == KD - 1))
                    grt = start // P + rt
                    nc.scalar.copy(
                        vnat[:, grt * NH * P:(grt + 1) * NH * P], vacc[:])
                # q,k matmuls: head-major m order so RoPE can start early
                for mt in (0, 4, 1, 5, 2, 6, 3, 7):
                    if ci == 0:
                        nc.sync.dma_start(
                            wqk_sb[:, mt * KD * P:(mt + 1) * KD * P],
                            wqk_in[mt].rearrange("p k m -> p (k m)"))
                    acc = pps.tile([P, nr], f32, tag="qkacc")
                    for kd in range(KD):
                        nc.tensor.matmul(
                            acc[:],
                            wqk_sb[:, (mt * KD + kd) * P:(mt * KD + kd + 1) * P],
                            xn1[:, kd * nr:(kd + 1) * nr],
                            start=(kd == 0), stop=(kd == KD - 1))
                    nc.scalar.activation(
                        qkT[:, mt * S + start: mt * S + start + nr],
                        acc[:], mybir.ActivationFunctionType.Identity,
                        bias=bqk_sb[:, mt:mt + 1])
            # rope constants, needed from phase B on
            nc.sync.dma_start(cosT[:], cosT_in)
            nc.sync.dma_start(sinTs[:], sinTs_in)
            nc.sync.dma_start(rotmT[:], rotmT_in)

        # ================= Phase B: attention (4 heads) =====================
        with tc.tile_pool(name="rope", bufs=2) as ropep, \
             tc.tile_pool(name="rtmp", bufs=2) as rtmpp, \
             tc.tile_pool(name="prT", bufs=4) as prtp, \
             tc.tile_pool(name="lsum", bufs=4) as lp, \
             tc.tile_pool(name="rbc", bufs=2) as rbcp, \
             tc.tile_pool(name="oT", bufs=2) as otp, \
             tc.tile_pool(name="sc_ps", bufs=2, space="PSUM") as scps, \
             tc.tile_pool(name="ov_ps", bufs=2, space="PSUM") as ovps, \
             tc.tile_pool(name="st_ps", bufs=2, space="PSUM") as stp2:
            def emit_norm(qc, opsum, lps, h, oTh):
                rw = lp.tile([1, 512], f32, tag="rinvw")
                nc.vector.reciprocal_approx_fast(rw[:], lps[:])
                rinv = lp.tile([1, 512], f32, tag="rinv")
                with nc.allow_low_precision(reason="f32r rounding for PE bcast"):
                    nc.vector.tensor_copy(rinv[:].bitcast(f32r), rw[:])
                rbc = scps.tile([P, 1024], f32, tag="scc")
                nc.tensor.matmul(rbc[:, 0:512], ones[0:1, :],
                                 rinv[:].bitcast(f32r),
                                 start=True, stop=True)
                rbs = rbcp.tile([P, 512], f32, tag="rbs")
                nc.vector.tensor_copy(rbs[:], rbc[:, 0:512])
                nc.vector.tensor_mul(oTh[:, qc * 512:(qc + 1) * 512],
                                     opsum[:], rbs[:])
                # stage the finished 256-column chunks for the AllToAll
                for j in (2 * qc, 2 * qc + 1):
                    nc.sync.dma_start(
                        a2a_in[h][j],
                        oTh[:, j * RH2:(j + 1) * RH2])

            def fire_norm(pend):
                qc, opsum, lps, h, oTh, last = pend
                emit_norm(qc, opsum, lps, h, oTh)
                if not last:
                    return
                # head h fully staged -> fire its AllToAll
                if single_core:
                    # timing-only stand-in for the collective (cost-model
                    # sim has no multi-core support)
                    nc.sync.dma_start(
                        a2a_out[h][:].rearrange("a r c -> (a r) c"),
                        a2a_in[h][:].rearrange("a r c -> (a r) c"))
                else:
                    nc.gpsimd.collective_compute(
                        "AllToAll", mybir.AluOpType.bypass,
                        replica_groups=GROUPS,
                        ins=[a2a_in[h][:].opt()],
                        outs=[a2a_out[h][:].opt()],
                    )

            pend_norm = None
            for h in range(NH):
                q_sl = qkT[:, h * S:(h + 1) * S]
                k_sl = qkT[:, (NH + h) * S:(NH + h + 1) * S]
                # RoPE on q and k. rotate_half is a cross-partition shuffle,
                # which DVE lanes cannot do, so apply it as a PE matmul with
                # a signed permutation matrix (sign of rotate_half baked in).
                rq = ropep.tile([P, S], bf16, tag="ropeq")
                rk = ropep.tile([P, S], bf16, tag="ropek")
                # per-512-chunk, k before q, so scores for q-chunk qc only
                # wait on rope chunks <= qc instead of the whole tensor
                for c in range(S // 512):
                    sl = slice(c * 512, (c + 1) * 512)
                    for src, dst in ((k_sl, rk), (q_sl, rq)):
                        tmp = rtmpp.tile([P, 512], bf16, tag="rtmp")
                        rt = scps.tile([P, 512], f32, tag="scc")
                        nc.tensor.matmul(
                            rt[:], rotmT[:], src[:, sl],
                            start=True, stop=True)
                        nc.vector.tensor_mul(tmp[:], rt[:], sinTs[:, sl])
                        nc.vector.tensor_mul(dst[:, sl], src[:, sl],
                                             cosT[:, sl])
                        nc.vector.tensor_add(dst[:, sl], dst[:, sl], tmp[:])
                oTh = otp.tile([P, S], bf16, tag="oTh")
                # q processed in 512-wide chunks; scores computed TRANSPOSED
                # (s.T[S_k, q]) so exp output is already in PV layout.
                # kt tiles are processed in PAIRS sharing one wide exp (the
                # ACT exp cadence, not the PE, limits this loop otherwise);
                # lps/opsum trail the scc/exp front by SKEWP pairs, and the
                # softmax normalization of chunk qc is deferred into chunk
                # qc+1 so the reciprocal latency hides behind matmuls.
                SKEWP = 2
                for qc in range(S // 512):
                    opsum = ovps.tile([P, 512], f32, tag="opv")
                    lps = stp2.tile([1, 512], f32, tag="lps")
                    npair = (4 * qc + 4) // 2
                    prts = [None] * npair

                    def front(pi, qc=qc, prts=prts):
                        scc = scps.tile([P, 1024], f32, tag="scc")
                        for half in range(2):
                            kt = 2 * pi + half
                            nc.tensor.matmul(
                                scc[:, half * 512:(half + 1) * 512],
                                rk[:, kt * P:(kt + 1) * P],
                                rq[:, qc * 512:(qc + 1) * 512],
                                start=True, stop=True)
                            lb = kt - 4 * qc
                            if lb >= 0:
                                # mask: lb full 128-blocks + triangular
                                nc.vector.tensor_add(
                                    scc[:, half * 512:half * 512 + (lb + 1) * P],
                                    scc[:, half * 512:half * 512 + (lb + 1) * P],
                                    wmask_sb[:, (3 - lb) * P:512])
                        prT = prtp.tile([P, 1024], bf16, tag="prT")
                        nc.scalar.activation(
                            prT[:], scc[:],
                            mybir.ActivationFunctionType.Exp, scale=SCALE)
                        prts[pi] = prT

                    def back(pi, npair=npair, lps=lps, opsum=opsum, h=h,
                             prts=prts):
                        prT = prts[pi]
                        for half in range(2):
                            kt = 2 * pi + half
                            nc.tensor.matmul(
                                lps[:], onesb[:, 0:1],
                                prT[:, half * 512:(half + 1) * 512],
                                start=(pi == 0 and half == 0),
                                stop=(pi == npair - 1 and half == 1))
                            nc.tensor.matmul(
                                opsum[:],
                                vnat[:, kt * NH * P + h * P:
                                     kt * NH * P + (h + 1) * P],
                                prT[:, half * 512:(half + 1) * 512],
                                start=(pi == 0 and half == 0),
                                stop=(pi == npair - 1 and half == 1))

                    for pi in range(npair):
                        front(pi)
                        if pi == SKEWP and pend_norm is not None:
                            fire_norm(pend_norm)
                            pend_norm = None
                        if pi >= SKEWP:
                            back(pi - SKEWP)
                    if pend_norm is not None:
                        fire_norm(pend_norm)
                        pend_norm = None
                    for pi in range(max(0, npair - SKEWP), npair):
                        back(pi)
                    pend_norm = (qc, opsum, lps, h, oTh, qc == S // 512 - 1)
            fire_norm(pend_norm)
            pend_norm = None
        qkv_stack.close()

        # ================= Phase C..F: row-parallel O-proj + MLP ============
        x1_stack = ExitStack()
        x1p = x1_stack.enter_context(tc.tile_pool(name="x1T", bufs=1))
        x1T = x1p.tile([P, KD * ROWS], f32, tag="x1T")
        omp = x1_stack.enter_context(tc.tile_pool(name="phC_om", bufs=1))
        oT_mine = omp.tile([P, KD * ROWS], bf16, tag="oT_mine")
        dps = x1_stack.enter_context(
            tc.tile_pool(name="phC_st", bufs=1, space="PSUM"))
        rp2 = x1_stack.enter_context(tc.tile_pool(name="phD_r", bufs=2))
        # pull the exchanged per-head chunks into SBUF as they land
        for h in range(NH):
            for j in range(N_CORES):
                b, g = divmod(j, 4)
                kd = 4 * g + h
                nc.sync.dma_start(
                    oT_mine[:, kd * ROWS + b * RH2: kd * ROWS + (b + 1) * RH2],
                    a2a_out[h][j])

        # k-tile order, h-major (matches the host-side permutation of wo's
        # k axis): tiles for head h arrive with AllToAll h
        PERM = [4 * g + h for h in range(NH) for g in range(NH)]
        with tc.tile_pool(name="phC_xr", bufs=3) as xrp, \
             tc.tile_pool(name="phC_w", bufs=3) as wop, \
             tc.tile_pool(name="phC_o1", bufs=1) as o1p, \
             tc.tile_pool(name="phC_sq", bufs=2) as sqp2, \
             tc.tile_pool(name="phC_ps", bufs=3, space="PSUM") as cps, \
             tc.tile_pool(name="phC_ps2", bufs=3, space="PSUM") as cps2:
            ssq2 = dps.tile([1, ROWS], f32, tag="ssq2")
            o1 = o1p.tile([P, KD * ROWS], f32, tag="o1")
            # pass 1: the 12 k-tiles from heads 0..2 — runnable while the
            # last head's AllToAll is still in flight
            NP1 = 12
            for mt in range(KD):
                wsb = wop.tile([P, NP1 * P], bf16, tag="wo1")
                nc.sync.dma_start(
                    wsb[:],
                    wo_in[mt][:, 0:NP1, :].rearrange("p k m -> p (k m)"))
                acc = cps.tile([P, ROWS], f32, tag="oacc1")
                for i in range(NP1):
                    kd = PERM[i]
                    nc.tensor.matmul(
                        acc[:], wsb[:, i * P:(i + 1) * P],
                        oT_mine[:, kd * ROWS:(kd + 1) * ROWS],
                        start=(i == 0), stop=(i == NP1 - 1))
                xr = xrp.tile([P, ROWS], f32, tag="xr")
                nc.sync.dma_start(xr[:], xrT_in[mt * P:(mt + 1) * P, :])
                # fold the residual in now, off the pass-2 critical path
                nc.vector.tensor_add(o1[:, mt * ROWS:(mt + 1) * ROWS],
                                     acc[:], xr[:])
            # pass 2: head 3's 4 k-tiles + residual + norm2 stats (squares on
            # DVE, PE sum trailing one tile)
            sq2s = [None] * KD
            for mt in range(KD):
                wsb = wop.tile([P, (KD - NP1) * P], bf16, tag="wo2")
                nc.sync.dma_start(
                    wsb[:],
                    wo_in[mt][:, NP1:KD, :].rearrange("p k m -> p (k m)"))
                acc = cps2.tile([P, ROWS], f32, tag="oacc2")
                for i in range(KD - NP1):
                    kd = PERM[NP1 + i]
                    nc.tensor.matmul(
                        acc[:], wsb[:, i * P:(i + 1) * P],
                        oT_mine[:, kd * ROWS:(kd + 1) * ROWS],
                        start=(i == 0), stop=(i == KD - NP1 - 1))
                nc.vector.tensor_add(x1T[:, mt * ROWS:(mt + 1) * ROWS],
                                     acc[:], o1[:, mt * ROWS:(mt + 1) * ROWS])
                sq = sqp2.tile([P, ROWS], bf16, tag="sq2")
                nc.scalar.activation(sq[:], x1T[:, mt * ROWS:(mt + 1) * ROWS],
                                     mybir.ActivationFunctionType.Square)
                sq2s[mt] = sq
                if mt >= 1:
                    nc.tensor.matmul(ssq2[:], onesb[:, 0:1], sq2s[mt - 1][:],
                                     start=(mt == 1), stop=False)
            nc.tensor.matmul(ssq2[:], onesb[:, 0:1], sq2s[KD - 1][:],
                             start=False, stop=True)

        sqr2 = rp2.tile([1, ROWS], f32, tag="sqr2")
        nc.scalar.activation(sqr2[:], ssq2[:],
                             mybir.ActivationFunctionType.Sqrt,
                             bias=eps_sb[0:1, :], scale=1.0 / D)
        rw2 = rp2.tile([1, ROWS], f32, tag="rw2")
        nc.vector.reciprocal_approx_fast(rw2[:], sqr2[:])
        rr2 = rp2.tile([1, ROWS], f32, tag="rr2")
        with nc.allow_low_precision(reason="f32r rounding for PE broadcast"):
            nc.vector.tensor_copy(rr2[:].bitcast(f32r), rw2[:])
        rb2 = dps.tile([P, ROWS], f32, tag="rb2")
        nc.tensor.matmul(rb2[:], ones[0:1, :],
                         rr2[:].bitcast(f32r), start=True, stop=True)

        mlp_stack = ExitStack()
        xn2p = mlp_stack.enter_context(tc.tile_pool(name="xn2", bufs=1))
        fnp = mlp_stack.enter_context(tc.tile_pool(name="fnT", bufs=1))
        xn2 = xn2p.tile([P, KD * ROWS], bf16, tag="xn2")
        fnT = fnp.tile([P, FT * ROWS], bf16, tag="fnT")
        for kd in range(KD):
            nc.vector.tensor_mul(xn2[:, kd * ROWS:(kd + 1) * ROWS],
                                 x1T[:, kd * ROWS:(kd + 1) * ROWS], rb2[:])

        with tc.tile_pool(name="phE_w", bufs=3) as wup_p, \
             tc.tile_pool(name="phE_sig", bufs=2) as sigp, \
             tc.tile_pool(name="phE_ps", bufs=4, space="PSUM") as eps_ps:
            for mt in range(FT):
                wsb = wup_p.tile([P, KD * P], bf16, tag="wup")
                nc.sync.dma_start(wsb[:], wup_in[mt].rearrange("p k m -> p (k m)"))
                acc = eps_ps.tile([P, ROWS], f32, tag="upacc")
                for kd in range(KD):
                    nc.tensor.matmul(
                        acc[:], wsb[:, kd * P:(kd + 1) * P],
                        xn2[:, kd * ROWS:(kd + 1) * ROWS],
                        start=(kd == 0), stop=(kd == KD - 1))
                sig = sigp.tile([P, ROWS], f32, tag="sig")
                nc.scalar.activation(sig[:], acc[:],
                                     mybir.ActivationFunctionType.Sigmoid,
                                     bias=bup_sb[:, mt:mt + 1])
                # fn = (up + b_up) * sigmoid(up + b_up), cast to bf16
                nc.vector.scalar_tensor_tensor(
                    fnT[:, mt * ROWS:(mt + 1) * ROWS], acc[:],
                    bup_sb[:, mt:mt + 1], sig[:],
                    op0=mybir.AluOpType.add, op1=mybir.AluOpType.mult)

        # down-proj: half-size weight tiles so the prefetch pipelines
        with tc.tile_pool(name="phF_w", bufs=3) as wdn_p, \
             tc.tile_pool(name="phF_out", bufs=2) as outp, \
             tc.tile_pool(name="phF_ps", bufs=4, space="PSUM") as fps:
            FH = FT // 2
            for mt in range(KD):
                acc = fps.tile([P, ROWS], f32, tag="dnacc")
                for half in range(2):
                    wsb = wdn_p.tile([P, FH * P], bf16, tag="wdn")
                    nc.sync.dma_start(
                        wsb[:],
                        wdn_in[mt][:, half * FH:(half + 1) * FH, :]
                        .rearrange("p k m -> p (k m)"))
                    for kt in range(FH):
                        kd = half * FH + kt
                        nc.tensor.matmul(
                            acc[:], wsb[:, kt * P:(kt + 1) * P],
                            fnT[:, kd * ROWS:(kd + 1) * ROWS],
                            start=(kd == 0), stop=(kd == FT - 1))
                out_sb = outp.tile([P, ROWS], f32, tag="out_sb")
                nc.vector.scalar_tensor_tensor(
                    out_sb[:], acc[:], bdn_sb[:, mt:mt + 1],
                    x1T[:, mt * ROWS:(mt + 1) * ROWS],
                    op0=mybir.AluOpType.add, op1=mybir.AluOpType.add)
                nc.sync.dma_start(outT[mt * P:(mt + 1) * P, :], out_sb[:])
        mlp_stack.close()
        x1_stack.close()


def host_prepare(inputs):
    """Fold LoRA/norm-weights/biases and build the 8 per-core input maps."""
    gi = {k: np.asarray(v, dtype=np.float32) if np.asarray(v).dtype != np.float32
          else np.asarray(v) for k, v in inputs.items()}

    def fold(nm):
        return gi['w_' + nm] + gi['w_' + nm + '_lora_a'] @ gi['w_' + nm + '_lora_b']

    nw1 = gi['norm_weight_1'][:, None]
    nw2 = gi['norm_weight_2'][:, None]
    w_q = (nw1 * fold('q')).astype(np.float32)
    w_k = (nw1 * fold('k')).astype(np.float32)
    w_v = (nw1 * fold('v')).astype(np.float32)
    w_o = fold('o').astype(np.float32)
    w_up = (nw2 * fold('up')).astype(np.float32)
    w_dn = fold('down').astype(np.float32)

    # pre-tiled weight layouts; wo's k axis is permuted h-major to match the
    # kernel's two-pass O-proj (tiles for head h arrive with AllToAll h)
    perm = [4 * g + h for h in range(4) for g in range(4)]
    wo_t = np.ascontiguousarray(
        w_o.reshape(KD, P, KD, P).transpose(2, 1, 0, 3)[:, :, perm, :]
    ).astype(ml_dtypes.bfloat16)
    wup_t = np.ascontiguousarray(
        w_up.reshape(KD, P, FT, P).transpose(2, 1, 0, 3)).astype(ml_dtypes.bfloat16)
    wdn_t = np.ascontiguousarray(
        w_dn.reshape(FT, P, KD, P).transpose(2, 1, 0, 3)).astype(ml_dtypes.bfloat16)
    bup_t = np.ascontiguousarray(gi['b_up'].reshape(FT, P).T)
    bdn_t = np.ascontiguousarray(gi['b_down'].reshape(KD, P).T)

    cosT = np.ascontiguousarray(gi['cos'].T).astype(ml_dtypes.bfloat16)
    sinTs = np.ascontiguousarray(gi['sin'].T).astype(ml_dtypes.bfloat16)
    # rot(x).T = R @ x.T with R[d, d+64] = -1 (d<64), R[d, d-64] = +1;
    # matmul computes lhsT.T @ rhs, so pass R.T.
    Rm = np.zeros((P, P), dtype=np.float32)
    hh = HD // 2
    Rm[np.arange(hh), np.arange(hh) + hh] = -1.0
    Rm[np.arange(hh) + hh, np.arange(hh)] = 1.0
    rotmT = np.ascontiguousarray(Rm.T).astype(ml_dtypes.bfloat16)
    maskT = np.maximum(gi['attention_mask'][0, 0, :P, :P], -2000.0).T
    wmask = np.full((P, 512), -2000.0, dtype=np.float32)
    wmask[:, 384:512] = maskT
    mask128 = np.ascontiguousarray(wmask)

    x = gi['x']
    # b_v and b_o folded into the residual: softmax rows sum to exactly 1,
    # so o_final = o @ w_o + (b_v @ w_o + b_o)
    radd = gi['b_v'] @ w_o + gi['b_o']

    # per-group (4-way TP) weight shards, shared across the two batches
    wqk_t, bqk_t, wv_t = [], [], []
    for g in range(4):
        hs = slice(512 * g, 512 * (g + 1))
        wqk = np.concatenate([w_q[:, hs], w_k[:, hs]], axis=1)
        wqk_t.append(np.ascontiguousarray(
            wqk.reshape(KD, P, MQK, P).transpose(2, 1, 0, 3)).astype(ml_dtypes.bfloat16))
        bqk = np.concatenate([gi['b_q'][hs], gi['b_k'][hs]])
        bqk_t.append(np.ascontiguousarray(bqk.reshape(MQK, P).T))
        wv_t.append(np.ascontiguousarray(
            w_v[:, hs].reshape(KD, P, NH * P).transpose(1, 0, 2)
            .reshape(P, KD * NH * P)).astype(ml_dtypes.bfloat16))

    xT = [np.ascontiguousarray(x[b].T).astype(ml_dtypes.bfloat16) for b in range(B)]

    ones = np.ones((P, P), dtype=np.float32)
    onesb = np.ones((P, P), dtype=ml_dtypes.bfloat16)

    in_maps = []
    for i in range(N_CORES):
        b, g = divmod(i, 4)
        # this core owns rows [256i, 256(i+1)) of BOTH batches
        xrows = np.concatenate(
            [x[0, RH2 * i:RH2 * (i + 1)], x[1, RH2 * i:RH2 * (i + 1)]], axis=0)
        xrT = np.ascontiguousarray(xrows.T + radd[:, None])
        in_maps.append({
            "xbT": xT[b], "xrT": xrT,
            "wqk": wqk_t[g], "bqk": bqk_t[g], "wv": wv_t[g],
            "wo": wo_t, "wup": wup_t, "bup": bup_t,
            "wdn": wdn_t, "bdn": bdn_t,
            "cosT": cosT, "sinTs": sinTs, "rotmT": rotmT,
            "ones": ones, "onesb": onesb, "mask": mask128,
        })
    return in_maps


def assemble(results):
    out = np.empty((B, S, D), dtype=np.float32)
    for i in range(N_CORES):
        oT = results[i]["outT"]
        out[0, RH2 * i:RH2 * (i + 1), :] = oT[:, 0:RH2].T
        out[1, RH2 * i:RH2 * (i + 1), :] = oT[:, RH2:ROWS].T
    return out


_NC_CACHE = {}


def get_nc():
    if "nc" not in _NC_CACHE:
        _NC_CACHE["nc"] = build_program()
    return _NC_CACHE["nc"]


def kernel(**inputs):
    nc = get_nc()
    in_maps = host_prepare(inputs)
    res = run_bass_kernel_spmd(nc, in_maps, list(range(N_CORES)))
    return assemble(res.results)
